# revision 9
# baseline (speedup 1.0000x reference)
"""Trainium2 Bass kernel for nn_MoE_40372692582952.

MoE layer: B,S,D = 2,1024,768; E=8 experts (one per NeuronCore), I=384,
group-limited top-2 routing (4 groups of 2), shared expert, interleaved
swiglu with clip.

Strategy (expert-parallel, per the sharding hint):
  - Host computes the gate (softmax + group-limited top-k; ~25 MFLOP) and
    the per-expert token lists ("all-to-all by routed expert id" done as a
    host-side gather, since kernel() receives full inputs).
  - Core c holds expert c's weights. It runs the expert FFN densely over
    the tokens routed to expert c (padded to CAP), and the shared-expert
    FFN over its 1/8 token slice (shared weights replicated).
  - Activations are kept feature-major (transposed) on device so every
    matmul contracts over the partition dim with no on-device transposes.
    swiglu's interleaved even/odd split is made contiguous by permuting
    the W1/W3/Ws1/Ws3 columns on the host (glu block first, linear block
    second).
  - Host scatter-adds the per-expert outputs (scaled by routing weights,
    plus B2/Bs2 biases, both folded on host) back into the full output.

Matmuls run as float32r (fp32 data, replicated-weight full-rate mode).
"""

import math
import sys

sys.path.insert(0, "/opt/trn_rl_repo")

import numpy as np

import concourse.bass as bass  # noqa: F401  (registers types)
import concourse.tile as tile
from concourse import bacc, mybir

# Model constants (hardcoded per spec).
B, S, D = 2, 1024, 768
I = 384
E = 8
G = 4
TOPK_GROUPS = 2
K = 2
ROUTE_SCALE = 1.0
ALPHA, LIMIT = 1.702, 7.0
T = B * S
NCORES = 8
SS = T // NCORES  # shared-expert token slice per core
P = 128
KD = D // P       # k-chunks over D
KI = I // P       # k-chunks over I
HT = (2 * I) // P  # h-tiles over 2I

# Column permutation: interleaved (even=glu, odd=linear) -> [glu | linear].
_PERM = np.concatenate([np.arange(0, 2 * I, 2), np.arange(1, 2 * I, 2)])

# Set by test harness: TRACE=True makes the SPMD run capture an NTFF trace.
TRACE = False
LAST_RESULTS = None

# Matmul input dtype: "f32r" (fp32 data, replicated full-rate mode) or
# "f16" (half DMA bytes for x/weights, ~5e-4 matmul rounding).
MM_DTYPE = "f32r"


def _mm_dt():
    return {"f32r": mybir.dt.float32r, "f16": mybir.dt.float16,
            "f32": mybir.dt.float32}[MM_DTYPE]


def _mm_np():
    return {"f32r": np.float32, "f16": np.float16, "f32": np.float32}[MM_DTYPE]


def _host_gate(xt, gate_w, gate_b):
    """Replicates reference.gate() in numpy fp32. Returns (w [T,K], idx [T,K])."""
    z = (xt @ gate_w.T.astype(np.float32)).astype(np.float32)
    z -= z.max(axis=-1, keepdims=True)
    ez = np.exp(z)
    scores = ez / ez.sum(axis=-1, keepdims=True)
    s = scores + gate_b[None, :].astype(np.float32)
    # top-2 of each group of 2 experts == sum of the group
    gs = s.reshape(T, G, E // G).sum(axis=-1)
    gidx = np.argsort(-gs, axis=-1, kind="stable")[:, :TOPK_GROUPS]
    keep = np.zeros((T, G), dtype=bool)
    np.put_along_axis(keep, gidx, True, axis=1)
    s_masked = np.where(np.repeat(keep, E // G, axis=1), s, -np.inf)
    idx = np.argsort(-s_masked, axis=-1, kind="stable")[:, :K]
    w = np.take_along_axis(scores, idx, axis=1) * ROUTE_SCALE
    return w.astype(np.float32), idx


def _ffn(nc, tc, pools, xT_ap, n_tok, tn, w1_s, w3_s, b1_s, b3_s, w2_s, out_ap):
    """Feature-major FFN: out[n_tok, D] = swiglu((xT^T@W1+b1)*(xT^T@W3+b3)) @ W2.

    xT_ap: DRAM [D, n_tok] (token-major transposed). out_ap: DRAM [n_tok, D].
    Weights already in SBUF, [128, KD, 2I] / [128, KI, D] layout, glu|lin
    permuted columns.
    """
    f32 = mybir.dt.float32
    mdt = _mm_dt()
    xp, hp, gp, op, pp1, pp3, ppy = pools
    ident = mybir.ActivationFunctionType.Identity
    sigm = mybir.ActivationFunctionType.Sigmoid
    alu = mybir.AluOpType

    xT_view = xT_ap.rearrange("(ko p) t -> p ko t", p=P)
    for tt in range(n_tok // tn):
        tsl = slice(tt * tn, (tt + 1) * tn)
        xT_t = xp.tile([P, KD, tn], mdt, tag="xT")
        nc.sync.dma_start(out=xT_t, in_=xT_view[:, :, tsl])

        ptiles = []
        for h in range(HT):
            ps1 = pp1.tile([P, tn], f32, tag="ps1")
            ps3 = pp3.tile([P, tn], f32, tag="ps3")
            hsl = slice(h * P, (h + 1) * P)
            for k in range(KD):
                nc.tensor.matmul(
                    ps1,
                    w1_s[:, k, hsl],
                    xT_t[:, k, :],
                    start=(k == 0),
                    stop=(k == KD - 1),
                )
            for k in range(KD):
                nc.tensor.matmul(
                    ps3,
                    w3_s[:, k, hsl],
                    xT_t[:, k, :],
                    start=(k == 0),
                    stop=(k == KD - 1),
                )
            # h1 = ps1 + b1 (ACT);  p = (ps3 + b3) * h1 (DVE, fused)
            h1t = hp.tile([P, tn], f32, tag="h1")
            nc.scalar.activation(out=h1t, in_=ps1, func=ident, bias=b1_s[:, h : h + 1])
            pt = hp.tile([P, tn], f32, tag=f"p{h}")
            nc.vector.scalar_tensor_tensor(
                out=pt, in0=ps3, scalar=b3_s[:, h : h + 1], in1=h1t,
                op0=alu.add, op1=alu.mult,
            )
            ptiles.append(pt)

        g_t = gp.tile([P, KI, tn], mdt, tag="g")
        for j in range(KI):
            pg, pl = ptiles[j], ptiles[j + KI]
            xg = hp.tile([P, tn], f32, tag="xg")
            nc.gpsimd.tensor_scalar_min(out=xg, in0=pg, scalar1=LIMIT)
            sg = hp.tile([P, tn], f32, tag="sg")
            nc.scalar.activation(out=sg, in_=xg, func=sigm, scale=ALPHA)
            m1 = hp.tile([P, tn], f32, tag="m1")
            nc.vector.tensor_mul(out=m1, in0=xg, in1=sg)
            xlc = hp.tile([P, tn], f32, tag="xlc")
            nc.gpsimd.tensor_scalar(
                out=xlc, in0=pl, scalar1=LIMIT, scalar2=-LIMIT,
                op0=alu.min, op1=alu.max,
            )
            # g = (xlc + 1) * (xg * sigmoid(alpha*xg))
            nc.vector.scalar_tensor_tensor(
                out=g_t[:, j, :], in0=xlc, scalar=1.0, in1=m1,
                op0=alu.add, op1=alu.mult,
            )

        DHALF = D // 2
        for t2 in range(tn // P):
            t2sl = slice(t2 * P, (t2 + 1) * P)
            ot = op.tile([P, D], f32, tag="out")
            for dc in range(2):
                dsl = slice(dc * DHALF, (dc + 1) * DHALF)
                psy = ppy.tile([P, DHALF], f32, tag="psy")
                for k in range(KI):
                    nc.tensor.matmul(
                        psy,
                        g_t[:, k, t2sl],
                        w2_s[:, k, dsl],
                        start=(k == 0),
                        stop=(k == KI - 1),
                    )
                nc.vector.tensor_copy(out=ot[:, dsl], in_=psy)
            nc.sync.dma_start(
                out=out_ap[tt * tn + t2 * P : tt * tn + (t2 + 1) * P, :], in_=ot
            )


_CACHE = {}


def _build(CAP, TN):
    key = (CAP, TN)
    if key in _CACHE:
        return _CACHE[key]
    f32 = mybir.dt.float32
    mdt = _mm_dt()
    nc = bacc.Bacc(None, target_bir_lowering=False, debug=False)

    xTg = nc.dram_tensor("xTg", [D, CAP], mdt, kind="ExternalInput")
    xTs = nc.dram_tensor("xTs", [D, SS], mdt, kind="ExternalInput")
    w1 = nc.dram_tensor("w1", [D, 2 * I], mdt, kind="ExternalInput")
    w3 = nc.dram_tensor("w3", [D, 2 * I], mdt, kind="ExternalInput")
    w2 = nc.dram_tensor("w2", [I, D], mdt, kind="ExternalInput")
    b1 = nc.dram_tensor("b1", [2 * I], f32, kind="ExternalInput")
    b3 = nc.dram_tensor("b3", [2 * I], f32, kind="ExternalInput")
    ws1 = nc.dram_tensor("ws1", [D, 2 * I], mdt, kind="ExternalInput")
    ws3 = nc.dram_tensor("ws3", [D, 2 * I], mdt, kind="ExternalInput")
    ws2 = nc.dram_tensor("ws2", [I, D], mdt, kind="ExternalInput")
    bs1 = nc.dram_tensor("bs1", [2 * I], f32, kind="ExternalInput")
    bs3 = nc.dram_tensor("bs3", [2 * I], f32, kind="ExternalInput")
    yg = nc.dram_tensor("yg", [CAP, D], f32, kind="ExternalOutput")
    ys = nc.dram_tensor("ys", [SS, D], f32, kind="ExternalOutput")

    with tile.TileContext(nc) as tc:
        with (
            tc.tile_pool(name="wts", bufs=1) as wp,
            tc.tile_pool(name="xin", bufs=2) as xp,
            tc.tile_pool(name="hbuf", bufs=2) as hp,
            tc.tile_pool(name="gbuf", bufs=2) as gp,
            tc.tile_pool(name="obuf", bufs=3) as op,
            tc.tile_pool(name="ps1", bufs=2, space="PSUM") as pp1,
            tc.tile_pool(name="ps3", bufs=2, space="PSUM") as pp3,
            tc.tile_pool(name="psy", bufs=2, space="PSUM") as ppy,
        ):
            def load_w(t, ko, m, name):
                s = wp.tile([P, ko, m], mdt, tag=name)
                nc.sync.dma_start(
                    out=s, in_=t.ap().rearrange("(ko p) m -> p ko m", p=P)
                )
                return s

            def load_b(t, name):
                s = wp.tile([P, HT], f32, tag=name)
                nc.sync.dma_start(out=s, in_=t.ap().rearrange("(o p) -> p o", p=P))
                return s

            w1_s = load_w(w1, KD, 2 * I, "w1")
            w3_s = load_w(w3, KD, 2 * I, "w3")
            w2_s = load_w(w2, KI, D, "w2")
            b1_s = load_b(b1, "b1")
            b3_s = load_b(b3, "b3")
            ws1_s = load_w(ws1, KD, 2 * I, "ws1")
            ws3_s = load_w(ws3, KD, 2 * I, "ws3")
            ws2_s = load_w(ws2, KI, D, "ws2")
            bs1_s = load_b(bs1, "bs1")
            bs3_s = load_b(bs3, "bs3")

            pools = (xp, hp, gp, op, pp1, pp3, ppy)
            _ffn(nc, tc, pools, xTg.ap(), CAP, TN, w1_s, w3_s, b1_s, b3_s, w2_s,
                 yg.ap())
            _ffn(nc, tc, pools, xTs.ap(), SS, 256, ws1_s, ws3_s, bs1_s, bs3_s,
                 ws2_s, ys.ap())

    nc.compile()
    _CACHE[key] = nc
    return nc


def _pick_cap(max_count):
    cap0 = max(256, int(math.ceil(max_count / P)) * P)
    best = None
    for tn in (512, 384, 256):
        cap = int(math.ceil(cap0 / tn)) * tn
        if best is None or cap < best[0]:
            best = (cap, tn)
    return best


def kernel(x, gate_w, gate_b, W1, B1, W3, B3, W2, B2,
           Ws1, Bs1, Ws3, Bs3, Ws2, Bs2):
    global LAST_RESULTS
    f = np.float32
    x = np.asarray(x, f)
    xt = np.ascontiguousarray(x.reshape(T, D))
    gate_w = np.asarray(gate_w, f)
    gate_b = np.asarray(gate_b, f)
    W1, B1 = np.asarray(W1, f), np.asarray(B1, f)
    W3, B3 = np.asarray(W3, f), np.asarray(B3, f)
    W2, B2 = np.asarray(W2, f), np.asarray(B2, f)
    Ws1, Bs1 = np.asarray(Ws1, f), np.asarray(Bs1, f)
    Ws3, Bs3 = np.asarray(Ws3, f), np.asarray(Bs3, f)
    Ws2, Bs2 = np.asarray(Ws2, f), np.asarray(Bs2, f)

    w, idx = _host_gate(xt, gate_w, gate_b)

    toks, wts = [], []
    for e in range(E):
        mask = (idx == e).any(axis=1)
        te = np.nonzero(mask)[0]
        ke = (idx[te] == e).argmax(axis=1)
        toks.append(te)
        wts.append(w[te, ke])
    counts = [len(t) for t in toks]
    CAP, TN = _pick_cap(max(counts))

    nc = _build(CAP, TN)

    # Shared-weight tensors are identical across cores; permute once.
    mnp = _mm_np()
    ws1_p = np.ascontiguousarray(Ws1[:, _PERM]).astype(mnp)
    ws3_p = np.ascontiguousarray(Ws3[:, _PERM]).astype(mnp)
    bs1_p = np.ascontiguousarray(Bs1[_PERM])
    bs3_p = np.ascontiguousarray(Bs3[_PERM])
    ws2_c = np.ascontiguousarray(Ws2).astype(mnp)

    in_maps = []
    for c in range(NCORES):
        xTg = np.zeros((D, CAP), mnp)
        xTg[:, : counts[c]] = xt[toks[c]].T
        xTs = np.ascontiguousarray(xt[c * SS : (c + 1) * SS].T).astype(mnp)
        in_maps.append({
            "xTg": xTg,
            "xTs": xTs,
            "w1": np.ascontiguousarray(W1[c][:, _PERM]).astype(mnp),
            "w3": np.ascontiguousarray(W3[c][:, _PERM]).astype(mnp),
            "w2": np.ascontiguousarray(W2[c]).astype(mnp),
            "b1": np.ascontiguousarray(B1[c][_PERM]),
            "b3": np.ascontiguousarray(B3[c][_PERM]),
            "ws1": ws1_p,
            "ws3": ws3_p,
            "ws2": ws2_c,
            "bs1": bs1_p,
            "bs3": bs3_p,
        })

    from concourse.bass_utils import run_bass_kernel_spmd

    kwargs = {}
    if TRACE:
        kwargs["tmpdir"] = "/tmp/moe_trace"
        import os
        import shutil

        shutil.rmtree("/tmp/moe_trace", ignore_errors=True)
        os.makedirs("/tmp/moe_trace", exist_ok=True)
    res = run_bass_kernel_spmd(
        nc, in_maps, core_ids=list(range(NCORES)), trace=TRACE, **kwargs
    )
    LAST_RESULTS = res

    out = np.zeros((T, D), f)
    for c in range(NCORES):
        ygc = np.asarray(res.results[c]["yg"], f)
        out[toks[c]] += wts[c][:, None] * (ygc[: counts[c]] + B2[c][None, :])
    for c in range(NCORES):
        ysc = np.asarray(res.results[c]["ys"], f)
        out[c * SS : (c + 1) * SS] += ysc + Bs2[None, :]
    return out.reshape(B, S, D)


# revision 12
# speedup vs baseline: 1.6578x; 1.6578x over previous
"""Trainium2 Bass kernel for nn_MoE_40372692582952.

MoE layer: B,S,D = 2,1024,768; E=8 experts (one per NeuronCore), I=384,
group-limited top-2 routing (4 groups of 2), shared expert, interleaved
swiglu with clip.

Strategy (expert-parallel, per the sharding hint):
  - Host computes the gate (softmax + group-limited top-k; ~25 MFLOP) and
    the per-expert token lists ("all-to-all by routed expert id" done as a
    host-side gather, since kernel() receives full inputs).
  - Core c holds expert c's weights. It runs the expert FFN densely over
    the tokens routed to expert c (padded to CAP), and the shared-expert
    FFN over its 1/8 token slice (shared weights replicated).
  - Activations are kept feature-major (transposed) on device so every
    matmul contracts over the partition dim with no on-device transposes.
    swiglu's interleaved even/odd split is made contiguous by permuting
    the W1/W3/Ws1/Ws3 columns on the host (glu block first, linear block
    second).
  - Host scatter-adds the per-expert outputs (scaled by routing weights,
    plus B2/Bs2 biases, both folded on host) back into the full output.

Matmuls run as float32r (fp32 data, replicated-weight full-rate mode).
"""

import math
import sys

sys.path.insert(0, "/opt/trn_rl_repo")

import numpy as np

import concourse.bass as bass  # noqa: F401  (registers types)
import concourse.tile as tile
from concourse import bacc, mybir

# Model constants (hardcoded per spec).
B, S, D = 2, 1024, 768
I = 384
E = 8
G = 4
TOPK_GROUPS = 2
K = 2
ROUTE_SCALE = 1.0
ALPHA, LIMIT = 1.702, 7.0
T = B * S
NCORES = 8
SS = T // NCORES  # shared-expert token slice per core
P = 128
KD = D // P       # k-chunks over D
KI = I // P       # k-chunks over I
HT = (2 * I) // P  # h-tiles over 2I

# Column permutation: interleaved (even=glu, odd=linear) -> [glu | linear].
_PERM = np.concatenate([np.arange(0, 2 * I, 2), np.arange(1, 2 * I, 2)])

# Set by test harness: TRACE=True makes the SPMD run capture an NTFF trace.
TRACE = False
LAST_RESULTS = None

# Matmul input dtype: "f32r" (fp32 data, replicated full-rate mode) or
# "f16" (half DMA bytes for x/weights, ~5e-4 matmul rounding).
MM_DTYPE = "f32r"


def _mm_dt():
    return {"f32r": mybir.dt.float32r, "f16": mybir.dt.float16,
            "f32": mybir.dt.float32}[MM_DTYPE]


def _mm_np():
    return {"f32r": np.float32, "f16": np.float16, "f32": np.float32}[MM_DTYPE]


def _host_gate(xt, gate_w, gate_b):
    """Replicates reference.gate() in numpy fp32. Returns (w [T,K], idx [T,K])."""
    z = (xt @ gate_w.T.astype(np.float32)).astype(np.float32)
    z -= z.max(axis=-1, keepdims=True)
    ez = np.exp(z)
    scores = ez / ez.sum(axis=-1, keepdims=True)
    s = scores + gate_b[None, :].astype(np.float32)
    # top-2 of each group of 2 experts == sum of the group
    gs = s.reshape(T, G, E // G).sum(axis=-1)
    gidx = np.argsort(-gs, axis=-1, kind="stable")[:, :TOPK_GROUPS]
    keep = np.zeros((T, G), dtype=bool)
    np.put_along_axis(keep, gidx, True, axis=1)
    s_masked = np.where(np.repeat(keep, E // G, axis=1), s, -np.inf)
    idx = np.argsort(-s_masked, axis=-1, kind="stable")[:, :K]
    w = np.take_along_axis(scores, idx, axis=1) * ROUTE_SCALE
    return w.astype(np.float32), idx


def _ffn(nc, tc, pools, xT_ap, n_tok, tn, w1_s, w3_s, b1_s, b3_s, w2_s, out_ap):
    """Feature-major FFN: out[n_tok, D] = swiglu((xT^T@W1+b1)*(xT^T@W3+b3)) @ W2.

    xT_ap: DRAM [D, n_tok] (token-major transposed). out_ap: DRAM [n_tok, D].
    Weights already in SBUF, [128, KD, 2I] / [128, KI, D] layout, glu|lin
    permuted columns.
    """
    f32 = mybir.dt.float32
    mdt = _mm_dt()
    xp, hp, gp, op, pp1, pp3, ppy = pools
    ident = mybir.ActivationFunctionType.Identity
    sigm = mybir.ActivationFunctionType.Sigmoid
    alu = mybir.AluOpType

    xT_view = xT_ap.rearrange("(ko p) t -> p ko t", p=P)
    for tt in range(n_tok // tn):
        tsl = slice(tt * tn, (tt + 1) * tn)
        xT_t = xp.tile([P, KD, tn], mdt, tag="xT")
        # x loads ride the ACT HWDGE ring so they don't queue behind the
        # weight loads on the sync ring.
        nc.scalar.dma_start(out=xT_t, in_=xT_view[:, :, tsl])

        glup = hp.tile([P, KI, tn], f32, tag="glup")
        linp = hp.tile([P, KI, tn], f32, tag="linp")
        for h in range(HT):
            ps1 = pp1.tile([P, tn], f32, tag="ps1")
            ps3 = pp3.tile([P, tn], f32, tag="ps3")
            hsl = slice(h * P, (h + 1) * P)
            for k in range(KD):
                nc.tensor.matmul(
                    ps1,
                    w1_s[:, k, hsl],
                    xT_t[:, k, :],
                    start=(k == 0),
                    stop=(k == KD - 1),
                )
            for k in range(KD):
                nc.tensor.matmul(
                    ps3,
                    w3_s[:, k, hsl],
                    xT_t[:, k, :],
                    start=(k == 0),
                    stop=(k == KD - 1),
                )
            # h1 = ps1 + b1 (ACT);  p = (ps3 + b3) * h1 (DVE, fused)
            h1t = hp.tile([P, tn], f32, tag="h1")
            nc.scalar.activation(out=h1t, in_=ps1, func=ident, bias=b1_s[:, h : h + 1])
            pt = glup[:, h, :] if h < KI else linp[:, h - KI, :]
            nc.vector.scalar_tensor_tensor(
                out=pt, in0=ps3, scalar=b3_s[:, h : h + 1], in1=h1t,
                op0=alu.add, op1=alu.mult,
            )

        # swiglu on the full [P, KI*tn] blocks (glu j pairs with lin j).
        # The sigmoid input skips the min-clip: sigmoid(1.702*x) is within
        # 7e-6 of its x=7 value beyond the clip point, and the exact
        # min(x,7) factor is applied in the fused multiplicand op below.
        g_t = gp.tile([P, KI, tn], mdt, tag="g")
        sg = hp.tile([P, KI, tn], f32, tag="sg")
        nc.scalar.activation(out=sg, in_=glup, func=sigm, scale=ALPHA)
        # m1 = min(glup,7) * sigmoid(alpha*glup)   (in place over sg)
        nc.vector.scalar_tensor_tensor(
            out=sg, in0=glup, scalar=LIMIT, in1=sg, op0=alu.min, op1=alu.mult,
        )
        # linp = clip(linp, -7, 7)   (in place)
        nc.vector.tensor_scalar(
            out=linp, in0=linp, scalar1=LIMIT, scalar2=-LIMIT,
            op0=alu.min, op1=alu.max,
        )
        # g = (linp + 1) * m1
        nc.vector.scalar_tensor_tensor(
            out=g_t, in0=linp, scalar=1.0, in1=sg, op0=alu.add, op1=alu.mult,
        )

        DHALF = D // 2
        for t2 in range(tn // P):
            t2sl = slice(t2 * P, (t2 + 1) * P)
            ot = op.tile([P, D], f32, tag="out")
            for dc in range(2):
                dsl = slice(dc * DHALF, (dc + 1) * DHALF)
                psy = ppy.tile([P, DHALF], f32, tag="psy")
                for k in range(KI):
                    nc.tensor.matmul(
                        psy,
                        g_t[:, k, t2sl],
                        w2_s[:, k, dsl],
                        start=(k == 0),
                        stop=(k == KI - 1),
                    )
                if dc == 0:
                    nc.vector.tensor_copy(out=ot[:, dsl], in_=psy)
                else:
                    nc.scalar.activation(
                        out=ot[:, dsl], in_=psy,
                        func=mybir.ActivationFunctionType.Copy,
                    )
            # output stores go via SWDGE (gpsimd) to keep both HWDGE
            # rings free for loads.
            nc.gpsimd.dma_start(
                out=out_ap[tt * tn + t2 * P : tt * tn + (t2 + 1) * P, :], in_=ot
            )


_CACHE = {}


def _build(CAP, TN):
    key = (CAP, TN)
    if key in _CACHE:
        return _CACHE[key]
    f32 = mybir.dt.float32
    mdt = _mm_dt()
    nc = bacc.Bacc(None, target_bir_lowering=False, debug=False)

    xTg = nc.dram_tensor("xTg", [D, CAP], mdt, kind="ExternalInput")
    xTs = nc.dram_tensor("xTs", [D, SS], mdt, kind="ExternalInput")
    w1 = nc.dram_tensor("w1", [D, 2 * I], mdt, kind="ExternalInput")
    w3 = nc.dram_tensor("w3", [D, 2 * I], mdt, kind="ExternalInput")
    w2 = nc.dram_tensor("w2", [I, D], mdt, kind="ExternalInput")
    b1 = nc.dram_tensor("b1", [2 * I], f32, kind="ExternalInput")
    b3 = nc.dram_tensor("b3", [2 * I], f32, kind="ExternalInput")
    ws1 = nc.dram_tensor("ws1", [D, 2 * I], mdt, kind="ExternalInput")
    ws3 = nc.dram_tensor("ws3", [D, 2 * I], mdt, kind="ExternalInput")
    ws2 = nc.dram_tensor("ws2", [I, D], mdt, kind="ExternalInput")
    bs1 = nc.dram_tensor("bs1", [2 * I], f32, kind="ExternalInput")
    bs3 = nc.dram_tensor("bs3", [2 * I], f32, kind="ExternalInput")
    yg = nc.dram_tensor("yg", [CAP, D], f32, kind="ExternalOutput")
    ys = nc.dram_tensor("ys", [SS, D], f32, kind="ExternalOutput")

    with tile.TileContext(nc) as tc:
        with (
            tc.tile_pool(name="wts", bufs=1) as wp,
            tc.tile_pool(name="xin", bufs=2) as xp,
            tc.tile_pool(name="hbuf", bufs=2) as hp,
            tc.tile_pool(name="gbuf", bufs=2) as gp,
            tc.tile_pool(name="obuf", bufs=3) as op,
            tc.tile_pool(name="ps1", bufs=2, space="PSUM") as pp1,
            tc.tile_pool(name="ps3", bufs=2, space="PSUM") as pp3,
            tc.tile_pool(name="psy", bufs=2, space="PSUM") as ppy,
        ):
            def load_w(t, ko, m, name):
                s = wp.tile([P, ko, m], mdt, tag=name)
                nc.sync.dma_start(
                    out=s, in_=t.ap().rearrange("(ko p) m -> p ko m", p=P)
                )
                return s

            def load_b(t, name):
                s = wp.tile([P, HT], f32, tag=name)
                nc.sync.dma_start(out=s, in_=t.ap().rearrange("(o p) -> p o", p=P))
                return s

            # Tiny bias loads first so they never gate an eviction; then
            # weights in first-use order so compute can start early.
            b1_s = load_b(b1, "b1")
            b3_s = load_b(b3, "b3")
            bs1_s = load_b(bs1, "bs1")
            bs3_s = load_b(bs3, "bs3")
            w1_s = load_w(w1, KD, 2 * I, "w1")
            w3_s = load_w(w3, KD, 2 * I, "w3")
            w2_s = load_w(w2, KI, D, "w2")
            ws1_s = load_w(ws1, KD, 2 * I, "ws1")
            ws3_s = load_w(ws3, KD, 2 * I, "ws3")
            ws2_s = load_w(ws2, KI, D, "ws2")

            pools = (xp, hp, gp, op, pp1, pp3, ppy)
            _ffn(nc, tc, pools, xTg.ap(), CAP, TN, w1_s, w3_s, b1_s, b3_s, w2_s,
                 yg.ap())
            _ffn(nc, tc, pools, xTs.ap(), SS, 256, ws1_s, ws3_s, bs1_s, bs3_s,
                 ws2_s, ys.ap())

    nc.compile()
    _CACHE[key] = nc
    return nc


def _pick_cap(max_count):
    cap0 = max(256, int(math.ceil(max_count / P)) * P)
    best = None
    for tn in (512, 384, 256):
        cap = int(math.ceil(cap0 / tn)) * tn
        if best is None or cap < best[0]:
            best = (cap, tn)
    return best


def kernel(x, gate_w, gate_b, W1, B1, W3, B3, W2, B2,
           Ws1, Bs1, Ws3, Bs3, Ws2, Bs2):
    global LAST_RESULTS
    f = np.float32
    x = np.asarray(x, f)
    xt = np.ascontiguousarray(x.reshape(T, D))
    gate_w = np.asarray(gate_w, f)
    gate_b = np.asarray(gate_b, f)
    W1, B1 = np.asarray(W1, f), np.asarray(B1, f)
    W3, B3 = np.asarray(W3, f), np.asarray(B3, f)
    W2, B2 = np.asarray(W2, f), np.asarray(B2, f)
    Ws1, Bs1 = np.asarray(Ws1, f), np.asarray(Bs1, f)
    Ws3, Bs3 = np.asarray(Ws3, f), np.asarray(Bs3, f)
    Ws2, Bs2 = np.asarray(Ws2, f), np.asarray(Bs2, f)

    w, idx = _host_gate(xt, gate_w, gate_b)

    toks, wts = [], []
    for e in range(E):
        mask = (idx == e).any(axis=1)
        te = np.nonzero(mask)[0]
        ke = (idx[te] == e).argmax(axis=1)
        toks.append(te)
        wts.append(w[te, ke])
    counts = [len(t) for t in toks]
    CAP, TN = _pick_cap(max(counts))

    nc = _build(CAP, TN)

    # Shared-weight tensors are identical across cores; permute once.
    mnp = _mm_np()
    ws1_p = np.ascontiguousarray(Ws1[:, _PERM]).astype(mnp)
    ws3_p = np.ascontiguousarray(Ws3[:, _PERM]).astype(mnp)
    bs1_p = np.ascontiguousarray(Bs1[_PERM])
    bs3_p = np.ascontiguousarray(Bs3[_PERM])
    ws2_c = np.ascontiguousarray(Ws2).astype(mnp)

    in_maps = []
    for c in range(NCORES):
        xTg = np.zeros((D, CAP), mnp)
        xTg[:, : counts[c]] = xt[toks[c]].T
        xTs = np.ascontiguousarray(xt[c * SS : (c + 1) * SS].T).astype(mnp)
        in_maps.append({
            "xTg": xTg,
            "xTs": xTs,
            "w1": np.ascontiguousarray(W1[c][:, _PERM]).astype(mnp),
            "w3": np.ascontiguousarray(W3[c][:, _PERM]).astype(mnp),
            "w2": np.ascontiguousarray(W2[c]).astype(mnp),
            "b1": np.ascontiguousarray(B1[c][_PERM]),
            "b3": np.ascontiguousarray(B3[c][_PERM]),
            "ws1": ws1_p,
            "ws3": ws3_p,
            "ws2": ws2_c,
            "bs1": bs1_p,
            "bs3": bs3_p,
        })

    from concourse.bass_utils import run_bass_kernel_spmd

    kwargs = {}
    if TRACE:
        kwargs["tmpdir"] = "/tmp/moe_trace"
        import os
        import shutil

        shutil.rmtree("/tmp/moe_trace", ignore_errors=True)
        os.makedirs("/tmp/moe_trace", exist_ok=True)
    res = run_bass_kernel_spmd(
        nc, in_maps, core_ids=list(range(NCORES)), trace=TRACE, **kwargs
    )
    LAST_RESULTS = res

    out = np.zeros((T, D), f)
    for c in range(NCORES):
        ygc = np.asarray(res.results[c]["yg"], f)
        out[toks[c]] += wts[c][:, None] * (ygc[: counts[c]] + B2[c][None, :])
    for c in range(NCORES):
        ysc = np.asarray(res.results[c]["ys"], f)
        out[c * SS : (c + 1) * SS] += ysc + Bs2[None, :]
    return out.reshape(B, S, D)


# revision 18
# speedup vs baseline: 1.7859x; 1.0773x over previous
"""Trainium2 Bass kernel for nn_MoE_40372692582952.

MoE layer: B,S,D = 2,1024,768; E=8 experts (one per NeuronCore), I=384,
group-limited top-2 routing (4 groups of 2), shared expert, interleaved
swiglu with clip.

Strategy (expert-parallel, per the sharding hint):
  - Host computes the gate (softmax + group-limited top-k; ~25 MFLOP) and
    the per-expert token lists ("all-to-all by routed expert id" done as a
    host-side gather, since kernel() receives full inputs).
  - Core c holds expert c's weights. It runs the expert FFN densely over
    the tokens routed to expert c (padded to CAP), and the shared-expert
    FFN over its 1/8 token slice (shared weights replicated).
  - Activations are kept feature-major (transposed) on device so every
    matmul contracts over the partition dim with no on-device transposes.
    swiglu's interleaved even/odd split is made contiguous by permuting
    the W1/W3/Ws1/Ws3 columns on the host (glu block first, linear block
    second).
  - Host scatter-adds the per-expert outputs (scaled by routing weights,
    plus B2/Bs2 biases, both folded on host) back into the full output.

Matmuls run as float32r (fp32 data, replicated-weight full-rate mode).
"""

import math
import sys

sys.path.insert(0, "/opt/trn_rl_repo")

import numpy as np

import concourse.bass as bass  # noqa: F401  (registers types)
import concourse.tile as tile
from concourse import bacc, mybir

# Model constants (hardcoded per spec).
B, S, D = 2, 1024, 768
I = 384
E = 8
G = 4
TOPK_GROUPS = 2
K = 2
ROUTE_SCALE = 1.0
ALPHA, LIMIT = 1.702, 7.0
T = B * S
NCORES = 8
SS = T // NCORES  # shared-expert token slice per core
STN = 256         # shared-phase token tile
P = 128
KD = D // P       # k-chunks over D
KI = I // P       # k-chunks over I
HT = (2 * I) // P  # h-tiles over 2I

# Column permutation: interleaved (even=glu, odd=linear) -> [glu | linear].
_PERM = np.concatenate([np.arange(0, 2 * I, 2), np.arange(1, 2 * I, 2)])

# Set by test harness: TRACE=True makes the SPMD run capture an NTFF trace.
TRACE = False
LAST_RESULTS = None

# Matmul input dtype: "f32r" (fp32 data, replicated full-rate mode) or
# "f16" (half DMA bytes for x/weights, ~5e-4 matmul rounding).
MM_DTYPE = "f32r"


def _mm_dt():
    return {"f32r": mybir.dt.float32r, "f16": mybir.dt.float16,
            "f32": mybir.dt.float32}[MM_DTYPE]


def _mm_np():
    return {"f32r": np.float32, "f16": np.float16, "f32": np.float32}[MM_DTYPE]


def _host_gate(xt, gate_w, gate_b):
    """Replicates reference.gate() in numpy fp32. Returns (w [T,K], idx [T,K])."""
    z = (xt @ gate_w.T.astype(np.float32)).astype(np.float32)
    z -= z.max(axis=-1, keepdims=True)
    ez = np.exp(z)
    scores = ez / ez.sum(axis=-1, keepdims=True)
    s = scores + gate_b[None, :].astype(np.float32)
    # top-2 of each group of 2 experts == sum of the group
    gs = s.reshape(T, G, E // G).sum(axis=-1)
    gidx = np.argsort(-gs, axis=-1, kind="stable")[:, :TOPK_GROUPS]
    keep = np.zeros((T, G), dtype=bool)
    np.put_along_axis(keep, gidx, True, axis=1)
    s_masked = np.where(np.repeat(keep, E // G, axis=1), s, -np.inf)
    idx = np.argsort(-s_masked, axis=-1, kind="stable")[:, :K]
    w = np.take_along_axis(scores, idx, axis=1) * ROUTE_SCALE
    return w.astype(np.float32), idx


def _ffn(nc, tc, pools, xT_ap, n_tok, tn, xtag, w1_s, w3_s, b1_s, b3_s, w2_s,
         out_ap):
    """Feature-major FFN: out[n_tok, D] = swiglu((xT^T@W1+b1)*(xT^T@W3+b3)) @ W2.

    xT_ap: DRAM [n_tok//tn, P, KD, tn] (host pre-tiled, contiguous per tile).
    out_ap: DRAM [n_tok, D]. Weights already in SBUF, [P, KD, 2I] /
    [P, KI, D] layout, glu|lin permuted columns.
    """
    f32 = mybir.dt.float32
    mdt = _mm_dt()
    xp, hp, gp, op, pp1, pp3, ppy = pools
    ident = mybir.ActivationFunctionType.Identity
    sigm = mybir.ActivationFunctionType.Sigmoid
    alu = mybir.AluOpType

    for tt in range(n_tok // tn):
        xT_t = xp.tile([P, KD, tn], mdt, tag=xtag)
        # x loads ride the ACT HWDGE ring so they don't queue behind the
        # weight loads on the sync ring.
        nc.scalar.dma_start(out=xT_t, in_=xT_ap[tt])

        glup = hp.tile([P, KI, tn], f32, tag="glup")
        linp = hp.tile([P, KI, tn], f32, tag="linp")
        for h in range(HT):
            ps1 = pp1.tile([P, tn], f32, tag="ps1")
            ps3 = pp3.tile([P, tn], f32, tag="ps3")
            hsl = slice(h * P, (h + 1) * P)
            for k in range(KD):
                nc.tensor.matmul(
                    ps1,
                    w1_s[:, k, hsl],
                    xT_t[:, k, :],
                    start=(k == 0),
                    stop=(k == KD - 1),
                )
            for k in range(KD):
                nc.tensor.matmul(
                    ps3,
                    w3_s[:, k, hsl],
                    xT_t[:, k, :],
                    start=(k == 0),
                    stop=(k == KD - 1),
                )
            # h1 = ps1 + b1 (ACT);  p = (ps3 + b3) * h1 (DVE, fused)
            h1t = hp.tile([P, tn], f32, tag="h1")
            nc.scalar.activation(out=h1t, in_=ps1, func=ident, bias=b1_s[:, h : h + 1])
            pt = glup[:, h, :] if h < KI else linp[:, h - KI, :]
            nc.vector.scalar_tensor_tensor(
                out=pt, in0=ps3, scalar=b3_s[:, h : h + 1], in1=h1t,
                op0=alu.add, op1=alu.mult,
            )

        # swiglu on the full [P, KI*tn] blocks (glu j pairs with lin j).
        # The sigmoid input skips the min-clip: sigmoid(1.702*x) is within
        # 7e-6 of its x=7 value beyond the clip point, and the exact
        # min(x,7) factor is applied in the fused multiplicand op below.
        g_t = gp.tile([P, KI, tn], mdt, tag="g")
        sg = hp.tile([P, KI, tn], f32, tag="sg")
        nc.scalar.activation(out=sg, in_=glup, func=sigm, scale=ALPHA)
        # m1 = min(glup,7) * sigmoid(alpha*glup)   (in place over sg)
        nc.vector.scalar_tensor_tensor(
            out=sg, in0=glup, scalar=LIMIT, in1=sg, op0=alu.min, op1=alu.mult,
        )
        # linp = clip(linp, -7, 7)   (in place)
        nc.vector.tensor_scalar(
            out=linp, in0=linp, scalar1=LIMIT, scalar2=-LIMIT,
            op0=alu.min, op1=alu.max,
        )
        # g = (linp + 1) * m1
        nc.vector.scalar_tensor_tensor(
            out=g_t, in0=linp, scalar=1.0, in1=sg, op0=alu.add, op1=alu.mult,
        )

        DHALF = D // 2
        for t2 in range(tn // P):
            t2sl = slice(t2 * P, (t2 + 1) * P)
            ot = op.tile([P, D], f32, tag="out")
            for dc in range(2):
                dsl = slice(dc * DHALF, (dc + 1) * DHALF)
                psy = ppy.tile([P, DHALF], f32, tag="psy")
                for k in range(KI):
                    nc.tensor.matmul(
                        psy,
                        g_t[:, k, t2sl],
                        w2_s[:, k, dsl],
                        start=(k == 0),
                        stop=(k == KI - 1),
                    )
                if dc == 0:
                    nc.vector.tensor_copy(out=ot[:, dsl], in_=psy)
                else:
                    nc.scalar.activation(
                        out=ot[:, dsl], in_=psy,
                        func=mybir.ActivationFunctionType.Copy,
                    )
            # output stores go via SWDGE (gpsimd) to keep both HWDGE
            # rings free for loads.
            nc.gpsimd.dma_start(
                out=out_ap[tt * tn + t2 * P : tt * tn + (t2 + 1) * P, :], in_=ot
            )


_CACHE = {}


def _build(CAP, TN):
    key = (CAP, TN)
    if key in _CACHE:
        return _CACHE[key]
    f32 = mybir.dt.float32
    mdt = _mm_dt()
    nc = bacc.Bacc(None, target_bir_lowering=False, debug=False)

    # All inputs are host pre-tiled to [P, ...] contiguous layouts so each
    # DMA is 128 long contiguous per-partition runs (cheap descriptor issue).
    xTg = nc.dram_tensor("xTg", [CAP // TN, P, KD, TN], mdt, kind="ExternalInput")
    xTs = nc.dram_tensor("xTs", [SS // STN, P, KD, STN], mdt, kind="ExternalInput")
    w1 = nc.dram_tensor("w1", [P, KD, 2 * I], mdt, kind="ExternalInput")
    w3 = nc.dram_tensor("w3", [P, KD, 2 * I], mdt, kind="ExternalInput")
    w2 = nc.dram_tensor("w2", [P, KI, D], mdt, kind="ExternalInput")
    ws1 = nc.dram_tensor("ws1", [P, KD, 2 * I], mdt, kind="ExternalInput")
    ws3 = nc.dram_tensor("ws3", [P, KD, 2 * I], mdt, kind="ExternalInput")
    ws2 = nc.dram_tensor("ws2", [P, KI, D], mdt, kind="ExternalInput")
    # b1 | b3 | bs1 | bs3 side by side, [P, 4*HT]
    bcat = nc.dram_tensor("bcat", [P, 4 * HT], f32, kind="ExternalInput")
    yg = nc.dram_tensor("yg", [CAP, D], f32, kind="ExternalOutput")
    ys = nc.dram_tensor("ys", [SS, D], f32, kind="ExternalOutput")

    with tile.TileContext(nc) as tc:
        with (
            tc.tile_pool(name="wts", bufs=1) as wp,
            tc.tile_pool(name="xin", bufs=2) as xp,
            tc.tile_pool(name="hbuf", bufs=2) as hp,
            tc.tile_pool(name="gbuf", bufs=2) as gp,
            tc.tile_pool(name="obuf", bufs=2) as op,
            tc.tile_pool(name="ps1", bufs=2, space="PSUM") as pp1,
            tc.tile_pool(name="ps3", bufs=2, space="PSUM") as pp3,
            tc.tile_pool(name="psy", bufs=2, space="PSUM") as ppy,
        ):
            def load_w(t, ko, m, name):
                s = wp.tile([P, ko, m], mdt, tag=name)
                nc.sync.dma_start(out=s, in_=t.ap())
                return s

            # One tiny bias load, then weights in first-use order so
            # compute can start early.
            b_s = wp.tile([P, 4 * HT], f32, tag="bcat")
            nc.sync.dma_start(out=b_s, in_=bcat.ap())
            b1_s = b_s[:, 0:HT]
            b3_s = b_s[:, HT : 2 * HT]
            bs1_s = b_s[:, 2 * HT : 3 * HT]
            bs3_s = b_s[:, 3 * HT : 4 * HT]
            w1_s = load_w(w1, KD, 2 * I, "w1")
            w3_s = load_w(w3, KD, 2 * I, "w3")
            w2_s = load_w(w2, KI, D, "w2")
            ws1_s = load_w(ws1, KD, 2 * I, "ws1")
            ws3_s = load_w(ws3, KD, 2 * I, "ws3")
            ws2_s = load_w(ws2, KI, D, "ws2")

            pools = (xp, hp, gp, op, pp1, pp3, ppy)
            _ffn(nc, tc, pools, xTg.ap(), CAP, TN, "xT", w1_s, w3_s, b1_s,
                 b3_s, w2_s, yg.ap())
            _ffn(nc, tc, pools, xTs.ap(), SS, STN, "xTs", ws1_s, ws3_s, bs1_s,
                 bs3_s, ws2_s, ys.ap())

    nc.compile()
    _CACHE[key] = nc
    return nc


def _pick_cap(max_count):
    cap0 = max(256, int(math.ceil(max_count / P)) * P)
    best = None
    for tn in (512, 384, 256):
        cap = int(math.ceil(cap0 / tn)) * tn
        if best is None or cap < best[0]:
            best = (cap, tn)
    return best


def kernel(x, gate_w, gate_b, W1, B1, W3, B3, W2, B2,
           Ws1, Bs1, Ws3, Bs3, Ws2, Bs2):
    global LAST_RESULTS
    f = np.float32
    x = np.asarray(x, f)
    xt = np.ascontiguousarray(x.reshape(T, D))
    gate_w = np.asarray(gate_w, f)
    gate_b = np.asarray(gate_b, f)
    W1, B1 = np.asarray(W1, f), np.asarray(B1, f)
    W3, B3 = np.asarray(W3, f), np.asarray(B3, f)
    W2, B2 = np.asarray(W2, f), np.asarray(B2, f)
    Ws1, Bs1 = np.asarray(Ws1, f), np.asarray(Bs1, f)
    Ws3, Bs3 = np.asarray(Ws3, f), np.asarray(Bs3, f)
    Ws2, Bs2 = np.asarray(Ws2, f), np.asarray(Bs2, f)

    w, idx = _host_gate(xt, gate_w, gate_b)

    toks, wts = [], []
    for e in range(E):
        mask = (idx == e).any(axis=1)
        te = np.nonzero(mask)[0]
        ke = (idx[te] == e).argmax(axis=1)
        toks.append(te)
        wts.append(w[te, ke])
    counts = [len(t) for t in toks]
    CAP, TN = _pick_cap(max(counts))

    nc = _build(CAP, TN)

    mnp = _mm_np()

    def tile_w(W2d, ko):
        # [ko*P, M] -> [P, ko, M] (partition-major, contiguous)
        m = W2d.shape[1]
        return np.ascontiguousarray(
            W2d.reshape(ko, P, m).transpose(1, 0, 2)
        ).astype(mnp)

    def tile_x(rows, cap, tn):
        # token rows [n<=cap, D] -> [cap//tn, P, KD, tn]
        X = np.zeros((cap, D), np.float32)
        X[: len(rows)] = rows
        return np.ascontiguousarray(
            X.reshape(cap // tn, tn, KD, P).transpose(0, 3, 2, 1)
        ).astype(mnp)

    def col(Bv):
        return Bv.reshape(HT, P).T  # [P, HT]

    # Shared-weight tensors are identical across cores; permute once.
    ws1_p = tile_w(Ws1[:, _PERM], KD)
    ws3_p = tile_w(Ws3[:, _PERM], KD)
    ws2_c = tile_w(Ws2, KI)

    in_maps = []
    for c in range(NCORES):
        bcat = np.ascontiguousarray(np.concatenate(
            [col(B1[c][_PERM]), col(B3[c][_PERM]),
             col(Bs1[_PERM]), col(Bs3[_PERM])], axis=1
        ).astype(np.float32))
        in_maps.append({
            "xTg": tile_x(xt[toks[c]], CAP, TN),
            "xTs": tile_x(xt[c * SS : (c + 1) * SS], SS, STN),
            "w1": tile_w(W1[c][:, _PERM], KD),
            "w3": tile_w(W3[c][:, _PERM], KD),
            "w2": tile_w(W2[c], KI),
            "ws1": ws1_p,
            "ws3": ws3_p,
            "ws2": ws2_c,
            "bcat": bcat,
        })

    from concourse.bass_utils import run_bass_kernel_spmd

    kwargs = {}
    if TRACE:
        kwargs["tmpdir"] = "/tmp/moe_trace"
        import os
        import shutil

        shutil.rmtree("/tmp/moe_trace", ignore_errors=True)
        os.makedirs("/tmp/moe_trace", exist_ok=True)
    res = run_bass_kernel_spmd(
        nc, in_maps, core_ids=list(range(NCORES)), trace=TRACE, **kwargs
    )
    LAST_RESULTS = res

    out = np.zeros((T, D), f)
    for c in range(NCORES):
        ygc = np.asarray(res.results[c]["yg"], f)
        out[toks[c]] += wts[c][:, None] * (ygc[: counts[c]] + B2[c][None, :])
    for c in range(NCORES):
        ysc = np.asarray(res.results[c]["ys"], f)
        out[c * SS : (c + 1) * SS] += ysc + Bs2[None, :]
    return out.reshape(B, S, D)
